# revision 1
# baseline (speedup 1.0000x reference)
"""Chamfer-with-normals (6D NN search) Trainium2 kernel, I/O-optimized.

The axon-tunneled PJRT path costs ~70-80ms fixed per dispatch plus
~7-15ms/MB of host<->device traffic, which dwarfs the <1ms of actual
device compute.  This version therefore moves ALL pre/post work onto
the device and ships the minimum possible bytes:

  upload   1 tensor/core: ab [9, N+M]  (576KB f32 / 288KB f16 wire)
  download 1 tensor/core: res [128, 66] (33KB)

Device program per core (8 jobs = batch x direction, SPMD, no
collectives), operating on HALVED squared distances q' = -d/2:
  - setup: replicate ab into 4 partition bands; derive the payload
    matrix pay[point, (xyz,normal,count)] from ab's db columns via PE
    transposes (no separate upload); optional f16->f32 band converts.
  - Pass A: PE matmuls (K=8 aug vectors) + DVE max-reduce -> rowmax.
  - Interlude: PE transpose + ScalarE(-1) write -rowmax into row 8 of
    the query columns (DRAM roundtrip broadcast, as before).
  - Pass B: transposed K=9 recompute, z ~ 0 at the argmin; ACT
    Relu(BIG*z + 1) one-hot mask; payload matmuls accumulate selected
    db rows (xyz, normal, count) into PSUM, column-tiled 4-way.
  - Post (new, on device): PE-transpose the accumulator and the query
    columns into point-major tiles [128, 64*7]; a ~25-op DVE/ACT chain
    computes per-query xyz_dist, sign-invariant normal_dist, count;
    rows with count outside [0.95, 1.05] are masked out and the two
    good-row sums are reduced on device (ones-matmul + DVE reduce).
  - Output: res[:, 0:64] = per-query counts (block-major), res[0, 64:66]
    = (sum xyz_dist, sum normal_dist) over good rows.

Host: builds ab, launches one jitted shard_map call (output buffers are
created inside the jit - nothing but ab crosses the tunnel), recomputes
the rare flagged rows (ties / out-of-band) exactly in numpy, and
combines the means.

HW quirk handled: a PE LdWeights can carry at most ONE semaphore wait,
so tiny 1x1 "touch" matmuls absorb pending sems early and a transitive
reduction pass strips redundant waits afterwards.
"""

import sys

import numpy as np

for _p in ("/opt/trn_rl_repo", "/opt/pypackages"):
    if _p not in sys.path:
        sys.path.insert(0, _p)

B = 4
N = 8192  # queries per job
M = 8192  # database per job
P = 128
CH = 7  # payload channels: xyz(3), normal(3), count(1)
# One-hot band: pass-B z = q' - rowmax lands within +-ulp(q') of 0 at the
# argmax (same K-order products, single final rounding).  With halved
# distances |q'| <~ 30, ulp <~ 4e-6, so scale 2e5 (band 5e-6) keeps the
# argmax weight ~1 and everything with a distance gap > 1e-5 at weight 0.
# Count-channel outliers (ties, band misses) are recomputed on the host.
BIG = 2.0e5
EPS = 1e-12
WIRE_DT = "f16"  # "f32" | "f16" wire format for the ab upload

_PROG_CACHE = {}


def _build_program(n, m, wire_dt="f32", debug=False):
    import concourse.bass as bass
    import concourse.tile as tile
    from concourse import mybir
    from concourse.masks import make_identity
    from concourse.tile_rust import add_dep_helper

    f32 = mybir.dt.float32
    dt_in = f32 if wire_dt == "f32" else mybir.dt.float16
    nb = n // P  # query row blocks
    mb = m // P  # db row blocks
    n_chunks = n // 512
    m_chunks = m // 512
    acc_w = (n_chunks // 4) * 512  # col-tiled payload accumulator width
    nbq = n // P
    out_w = (3 * nbq + 2) if debug else (nbq + 2)

    nc = bass.Bass()
    ab_d = nc.dram_tensor("ab", [9, n + m], dt_in, kind="ExternalInput")
    out_d = nc.dram_tensor("res", [P, out_w], f32, kind="ExternalOutput")
    rmx_d = nc.dram_tensor("rmx", [n], f32)

    Act = mybir.ActivationFunctionType
    Alu = mybir.AluOpType

    with tile.TileContext(nc) as tc:
        with tc.tile_pool(name="singles", bufs=1) as singles:
            ab_sb = singles.tile([P, n + m], f32)
            if wire_dt != "f32":
                stage_sb = singles.tile([P, n + m], dt_in)
            pay_sb = singles.tile([P, mb * CH], f32)
            ident = singles.tile([P, P], f32)
            idb = singles.tile([P, 8], f32)  # per-band 7x7 identity
            rowmax = singles.tile([P, nb], f32)
            acc_sb = singles.tile([P, max(acc_w, P)], f32)
            # rmx_sb aliases the (yet-unwritten) acc_sb tile to avoid a fresh
            # SBUF region whose zone tracking would pull in unrelated DMA sems
            rmx_sb = acc_sb[0:nb, 0:P]
            ta_sb = singles.tile([P, nb * CH], f32)  # transposed payload acc
            tq_sb = singles.tile([P, nb * CH], f32)  # transposed query cols
            t1x = singles.tile([P, nb * 3], f32)
            t1n = singles.tile([P, nb * 3], f32)
            t2x = singles.tile([P, nb * 3], f32)
            t2n = singles.tile([P, nb * 3], f32)
            t3n = singles.tile([P, nb * 3], f32)
            qa = singles.tile([P, nb * 16], f32)  # chain quantities, 16 lanes
            gsum = singles.tile([P, 2 * nb], f32)
            ones_sb = singles.tile([P, 1], f32)
            out_sb = singles.tile([P, out_w], f32)

            def pe_touch(touch, ap, base=0):
                return nc.tensor.matmul(
                    out=touch[0:1, 0:1],
                    lhsT=ap,
                    rhs=ap,
                    start=True,
                    stop=True,
                    tile_position=(base, 0),
                )

            make_identity(nc, ident[:])
            nc.vector.memset(ones_sb[:], 1.0)
            nc.vector.memset(out_sb[:], 0.0)
            for r in range(4):
                if wire_dt == "f32":
                    nc.sync.dma_start(
                        out=ab_sb[32 * r : 32 * r + 9, :], in_=ab_d[:]
                    )
                else:
                    nc.sync.dma_start(
                        out=stage_sb[32 * r : 32 * r + 9, :], in_=ab_d[:]
                    )
                # band identity for the post-phase [7,128] transposes
                nc.sync.dma_start(
                    out=idb[32 * r : 32 * r + 7, 0:7], in_=ident[0:7, 0:7]
                )
            if wire_dt != "f32":
                for r in range(4):
                    eng = nc.scalar if r % 2 else nc.vector
                    if r % 2:
                        eng.activation(
                            out=ab_sb[32 * r : 32 * r + 9, :],
                            in_=stage_sb[32 * r : 32 * r + 9, :],
                            func=Act.Copy,
                        )
                    else:
                        eng.tensor_copy(
                            out=ab_sb[32 * r : 32 * r + 9, :],
                            in_=stage_sb[32 * r : 32 * r + 9, :],
                        )

            def a_g(r):
                return ab_sb[32 * r : 32 * r + 9, 0:n]

            def b_g(r):
                return ab_sb[32 * r : 32 * r + 9, n : n + m]

            with tc.tile_pool(name="touchps", bufs=1, space="PSUM") as tp0:
                touch0 = tp0.tile([1, 1], f32, space="PSUM")
                pe_touch(touch0, ident[0:1, 0:1])
                pe_touch(touch0, ones_sb[0:1, 0:1])
                pe_touch(touch0, idb[0:1, 0:1])
                for r in range(4):
                    pe_touch(
                        touch0, ab_sb[32 * r : 32 * r + 9, 0:1], base=32 * r
                    )

            # ------- payload derivation: pay[point, ch] from db columns ----
            pay3 = pay_sb[:, 0 : mb * CH].rearrange("p (b c) -> p b c", c=CH)
            with tc.tile_pool(name="trp", bufs=2, space="PSUM") as trp:
                for g in range(mb // 4):
                    tp4 = trp.tile([P, 24], f32, space="PSUM")
                    for s in range(4):
                        jb = 4 * g + s
                        nc.tensor.transpose(
                            out=tp4[:, 6 * s : 6 * s + 6],
                            in_=ab_sb[0:6, n + jb * P : n + (jb + 1) * P],
                            identity=ident[0:6, 0:6],
                        )
                    nc.scalar.activation(
                        out=pay3[:, 4 * g : 4 * g + 4, 0:6],
                        in_=tp4[:, 0:24].rearrange("p (b c) -> p b c", c=6),
                        func=Act.Copy,
                    )
                # count channel = 1.0
                nc.scalar.activation(
                    out=pay3[:, :, 6:7],
                    in_=ident[:, 0:mb].rearrange("p (b c) -> p b c", c=1),
                    func=Act.Copy,
                    scale=0.0,
                    bias=1.0,
                )

            with tc.tile_pool(name="touchps2", bufs=1, space="PSUM") as tpp:
                touchp = tpp.tile([1, 1], f32, space="PSUM")
                pe_touch(touchp, pay_sb[0:1, 0:1])

            # ---------------- Pass A: row maxima of q' ----------------
            with (
                tc.tile_pool(name="qps", bufs=2, space="PSUM") as qps,
                tc.tile_pool(name="rm", bufs=4) as rmpool,
            ):
                n_rounds = m_chunks // 4
                for ib in range(nb):
                    rm = rmpool.tile([P, n_rounds], f32)
                    for rnd in range(n_rounds):
                        q = qps.tile([P, 2048], f32, space="PSUM")
                        for r in range(4):
                            c = rnd * 4 + r
                            nc.tensor.matmul(
                                out=q[:, r * 512 : (r + 1) * 512],
                                lhsT=a_g(r)[0:8, ib * P : (ib + 1) * P],
                                rhs=b_g(r)[0:8, c * 512 : (c + 1) * 512],
                                start=True,
                                stop=True,
                                tile_position=(32 * r, 0),
                            )
                        nc.vector.tensor_reduce(
                            out=rm[:, rnd : rnd + 1],
                            in_=q[:, 0:2048],
                            axis=mybir.AxisListType.X,
                            op=Alu.max,
                        )
                    nc.vector.tensor_reduce(
                        out=rowmax[:, ib : ib + 1],
                        in_=rm[:, 0:n_rounds],
                        axis=mybir.AxisListType.X,
                        op=Alu.max,
                    )

            # transpose rowmax [P, nb] -> [nb, P], negate, roundtrip to
            # row 8 of bands 0/1 in natural i order.
            with (
                tc.tile_pool(name="rmxps", bufs=1, space="PSUM") as rmxps,
                tc.tile_pool(name="touchp2", bufs=1, space="PSUM") as tp2,
            ):
                rmx_ps = rmxps.tile([nb, P], f32, space="PSUM")
                nc.tensor.transpose(
                    out=rmx_ps[:], in_=rowmax[:, 0:nb], identity=ident[:]
                )
                nc.scalar.activation(
                    out=rmx_sb[:],
                    in_=rmx_ps[:],
                    func=Act.Copy,
                    scale=-1.0,
                )
                nc.sync.dma_start(
                    out=rmx_d[:].rearrange("(a b) -> a b", a=nb), in_=rmx_sb[:]
                )
                touch2 = tp2.tile([1, 1], f32, space="PSUM")
                row8_touches = []
                for r in range(2):
                    nc.sync.dma_start(
                        out=ab_sb[32 * r + 8 : 32 * r + 9, 0:n],
                        in_=rmx_d[None, :],
                    )
                    # absorb each band's row-8 DMA sem on PE (K=9 column
                    # overlapping row 8 at the band's base partition)
                    row8_touches.append(
                        pe_touch(
                            touch2,
                            ab_sb[32 * r : 32 * r + 9, 0:1],
                            base=32 * r,
                        )
                    )

            # ---------------- Pass B: mask + payload ----------------
            with (
                tc.tile_pool(name="zps", bufs=2, space="PSUM") as zps,
                tc.tile_pool(name="accps", bufs=1, space="PSUM") as accps,
                tc.tile_pool(name="mask", bufs=2) as maskpool,
            ):
                acc = accps.tile([P, acc_w], f32, space="PSUM")
                # absorb the acc-bank WAR handover on PE before the real
                # accumulation group opens (col-tiled like the payload mms)
                nc.tensor.matmul(
                    out=acc[0:1, 0:1],
                    lhsT=ab_sb[0:1, 0:1],
                    rhs=ab_sb[0:1, 0:1],
                    start=True,
                    stop=True,
                    tile_position=(0, 0),
                )
                # 2-chunk z rounds, double-buffered: ScalarE streams the
                # mask continuously instead of ping-ponging with PE.
                zb_rounds = n_chunks // 2
                _next_z_dep = {}
                for jb in range(mb):
                    mask = maskpool.tile([P, n], f32)
                    for rnd in range(zb_rounds):
                        z = zps.tile([P, 1024], f32, space="PSUM")
                        for r in range(2):
                            c = rnd * 2 + r
                            zmm = nc.tensor.matmul(
                                out=z[:, r * 512 : (r + 1) * 512],
                                lhsT=b_g(r)[:, jb * P : (jb + 1) * P],
                                rhs=a_g(r)[:, c * 512 : (c + 1) * 512],
                                start=True,
                                stop=True,
                                tile_position=(32 * r, 0),
                            )
                            if jb == 0 and rnd == 0:
                                add_dep_helper(
                                    zmm.ins,
                                    row8_touches[r].ins,
                                    reason="order row8 sem absorber first",
                                )
                            if rnd == 0 and r == 0 and jb in _next_z_dep:
                                add_dep_helper(
                                    zmm.ins,
                                    _next_z_dep[jb].ins,
                                    reason="group col-tiled payload mms",
                                )
                        nc.scalar.activation(
                            out=mask[:, rnd * 1024 : (rnd + 1) * 1024],
                            in_=z[:, 0:1024],
                            func=Act.Relu,
                            scale=BIG,
                            bias=1.0,
                        )
                    pay_first = None
                    for c in range(n_chunks):
                        pp = 32 * (c % 4)
                        fo = (c // 4) * 512
                        pmm = nc.tensor.matmul(
                            out=acc[pp : pp + CH, fo : fo + 512],
                            lhsT=pay_sb[:, jb * CH : (jb + 1) * CH],
                            rhs=mask[:, c * 512 : (c + 1) * 512],
                            start=(jb == 0),
                            stop=(jb == mb - 1),
                            tile_position=(0, pp),
                        )
                        if pay_first is None:
                            pay_first = pmm
                    if jb + 1 < mb:
                        _next_z_dep[jb + 1] = pay_first

                # ACT (not DVE) so the post-phase PE transposes see a single
                # Activation sem covering both this RAW and the PSUM-bank WAR
                nc.scalar.activation(
                    out=acc_sb[:, 0:acc_w], in_=acc[:], func=Act.Copy
                )

            # ---------------- Post: per-query metrics on device ----------
            ta3 = ta_sb[:, 0 : nb * CH].rearrange("p (b c) -> p b c", c=CH)
            tq3 = tq_sb[:, 0 : nb * CH].rearrange("p (b c) -> p b c", c=CH)
            with (
                tc.tile_pool(name="taps", bufs=2, space="PSUM") as taps,
                tc.tile_pool(name="tqps", bufs=2, space="PSUM") as tqps,
            ):
                # acc chunks live at (partition band 32*(c%4), free (c//4)*512)
                for c in range(n_chunks):
                    r = c % 4
                    fo = (c // 4) * 512
                    tpa = taps.tile([P, 28], f32, space="PSUM")
                    for s in range(4):
                        nc.tensor.transpose(
                            out=tpa[:, 7 * s : 7 * s + 7],
                            in_=acc_sb[
                                32 * r : 32 * r + CH,
                                fo + s * P : fo + (s + 1) * P,
                            ],
                            identity=idb[32 * r : 32 * r + 7, 0:7],
                            tile_position=(32 * r, 0),
                        )
                    nc.scalar.activation(
                        out=ta3[:, 4 * c : 4 * c + 4, :],
                        in_=tpa[:, 0:28].rearrange("p (b c) -> p b c", c=CH),
                        func=Act.Copy,
                    )
                for g in range(nb // 4):
                    tpq = tqps.tile([P, 28], f32, space="PSUM")
                    for s in range(4):
                        qb = 4 * g + s
                        nc.tensor.transpose(
                            out=tpq[:, 7 * s : 7 * s + 7],
                            in_=ab_sb[0:7, qb * P : (qb + 1) * P],
                            identity=idb[0:7, 0:7],
                        )
                    nc.scalar.activation(
                        out=tq3[:, 4 * g : 4 * g + 4, :],
                        in_=tpq[:, 0:28].rearrange("p (b c) -> p b c", c=CH),
                        func=Act.Copy,
                    )

            # chain quantities, each [P, nb] in lanes of qa
            def lane(i):
                return qa[:, i * nb : (i + 1) * nb]

            v3 = lambda t: t[:, 0 : nb * 3].rearrange("p (b c) -> p b c", c=3)
            vv = nc.vector
            # products
            vv.tensor_tensor(v3(t1x), tq3[:, :, 0:3], ta3[:, :, 0:3], Alu.mult)
            vv.tensor_tensor(v3(t1n), tq3[:, :, 3:6], ta3[:, :, 3:6], Alu.mult)
            vv.tensor_tensor(v3(t2x), ta3[:, :, 0:3], ta3[:, :, 0:3], Alu.mult)
            vv.tensor_tensor(v3(t2n), ta3[:, :, 3:6], ta3[:, :, 3:6], Alu.mult)
            vv.tensor_tensor(v3(t3n), tq3[:, :, 3:6], tq3[:, :, 3:6], Alu.mult)
            # reduces: A=q.t_xyz  D=a.t_n  B=|t_xyz|^2  E=|t_n|^2  Asq=|a|^2
            X = mybir.AxisListType.X
            vv.tensor_reduce(out=lane(0), in_=v3(t1x), axis=X, op=Alu.add)
            vv.tensor_reduce(out=lane(1), in_=v3(t1n), axis=X, op=Alu.add)
            vv.tensor_reduce(out=lane(2), in_=v3(t2x), axis=X, op=Alu.add)
            vv.tensor_reduce(out=lane(3), in_=v3(t2n), axis=X, op=Alu.add)
            vv.tensor_reduce(out=lane(4), in_=v3(t3n), axis=X, op=Alu.add)
            # scalar chain: DVE-only except the single ACT Sqrt, so every
            # instruction carries at most one cross-engine wait
            cntm = lane(5)
            vv.tensor_copy(
                out=cntm.rearrange("p (b c) -> p b c", c=1), in_=ta3[:, :, 6:7]
            )
            vv.tensor_scalar_max(lane(6), cntm, 0.5)
            vv.reciprocal(lane(7), lane(6))  # inv
            vv.tensor_tensor(lane(6), lane(7), lane(7), Alu.mult)  # inv^2
            vv.tensor_tensor(lane(8), lane(0), lane(7), Alu.mult)  # A*inv
            vv.tensor_tensor(lane(9), lane(2), lane(6), Alu.mult)  # B*inv^2
            # s1 = -2*A*inv + B*inv^2
            vv.scalar_tensor_tensor(
                lane(10), lane(8), -2.0, lane(9), Alu.mult, Alu.add
            )
            # xyz_d = (2*qsq_half + s1) - 1
            qsv = lane(13)
            vv.tensor_copy(
                out=qsv.rearrange("p (b c) -> p b c", c=1),
                in_=tq3[:, :, 6:7],
            )
            vv.scalar_tensor_tensor(
                lane(8), qsv, 2.0, lane(10), Alu.mult, Alu.add
            )
            xd = lane(11)
            vv.tensor_scalar_add(xd, lane(8), -1.0)
            # nd = |a|^2 + 1 - 2*|D|/sqrt(E)
            vv.tensor_scalar_max(lane(14), lane(3), 1e-12)
            nc.scalar.activation(out=lane(9), in_=lane(14), func=Act.Sqrt)
            vv.reciprocal(lane(10), lane(9))  # 1/sqrt(E); sole ACT->DVE wait
            vv.tensor_scalar_mul(lane(12), lane(1), -1.0)
            vv.tensor_tensor(lane(14), lane(1), lane(12), Alu.max)  # |D|
            vv.scalar_tensor_tensor(
                lane(12), lane(14), -2.0, lane(10), Alu.mult, Alu.mult
            )
            nd = lane(15)
            vv.scalar_tensor_tensor(
                nd, lane(4), 1.0, lane(12), Alu.add, Alu.add
            )
            # good-row mask ((cnt-1)^2 <= 0.0025; host mirrors bitwise)
            vv.tensor_scalar_add(lane(8), cntm, -1.0)
            vv.tensor_tensor(lane(6), lane(8), lane(8), Alu.mult)
            vv.tensor_scalar(lane(13), lane(6), 0.0025, None, Alu.is_le)
            vv.tensor_tensor(gsum[:, 0:nb], lane(13), xd, Alu.mult)
            vv.tensor_tensor(gsum[:, nb : 2 * nb], lane(13), nd, Alu.mult)
            with tc.tile_pool(name="sums", bufs=1, space="PSUM") as sps:
                tch = sps.tile([1, 1], f32, space="PSUM")
                pe_touch(tch, ones_sb[0:1, 0:1])
                sums_ps = sps.tile([1, 2 * nb], f32, space="PSUM")
                nc.tensor.matmul(
                    out=sums_ps[:],
                    lhsT=ones_sb[:, 0:1],
                    rhs=gsum[:, 0 : 2 * nb],
                    start=True,
                    stop=True,
                    tile_position=(0, 0),
                )
                so = 3 * nb if debug else nb
                vv.tensor_reduce(
                    out=out_sb[0:1, so : so + 2],
                    in_=sums_ps[0:1, :].rearrange("p (a b) -> p a b", a=2),
                    axis=X,
                    op=Alu.add,
                )
            # all out_sb writers on DVE so the final DMA carries one wait
            vv.tensor_copy(out=out_sb[:, 0:nb], in_=cntm)
            if debug:
                vv.tensor_copy(out=out_sb[:, nb : 2 * nb], in_=xd)
                vv.tensor_copy(out=out_sb[:, 2 * nb : 3 * nb], in_=nd)
            nc.sync.dma_start(out=out_d[:], in_=out_sb[:, 0:out_w])

    _strip_redundant_pe_waits(nc)
    return nc


def _strip_redundant_pe_waits(nc):
    """Drop transitively-redundant semaphore waits from PE instructions.

    A PE LdWeights can carry only ONE sync wait, but Tile's sem assignment
    is not transitively minimal: a matmul often gets both a PE self-wait
    (PSUM WAW) and a DVE/ACT wait (WAR) where the latter already implies the
    former (the consumer that frees the PSUM slot itself waited on the PE
    writes).  Soundness: sem >= v means the instructions contributing the
    first v increments have *completed*, hence their own waits were
    satisfied, recursively.
    """
    f = nc.m.functions[0]
    insts = [ins for bb in f.blocks for ins in bb.instructions]

    sem_incs = {}  # sem id -> list of (cum_value, inst_idx)
    for k, ins in enumerate(insts):
        si = ins.sync_info
        if si is None:
            continue
        for up in si.on_update:
            if up.sync_type != "semaphore" or up.update_mode not in (
                "sem-inc",
                "sem-add-imm",
            ):
                continue
            lst = sem_incs.setdefault(up.id, [])
            prev = lst[-1][0] if lst else 0
            lst.append((prev + up.update_value, k))

    closure_memo = {}
    prefix_memo = {}

    def merge(dst, src):
        for s, v in src.items():
            if dst.get(s, -1) < v:
                dst[s] = v

    def closure(k):
        got = closure_memo.get(k)
        if got is not None:
            return got
        closure_memo[k] = {}  # cycle guard
        out = {}
        si = insts[k].sync_info
        if si is not None:
            for w in si.on_wait:
                if (
                    w.sync_type == "semaphore"
                    and w.wait_mode == "sem-ge-imm"
                    and w.wait_reg is None
                ):
                    merge(out, wait_implies(w.id, w.wait_value))
        closure_memo[k] = out
        return out

    def wait_implies(semid, v):
        out = {semid: v}
        lst = sem_incs.get(semid, [])
        if semid not in prefix_memo:
            prefix_memo[semid] = []
        prefs = prefix_memo[semid]
        while True:
            idx = len(prefs)
            if idx >= len(lst) or lst[idx][0] > v:
                break
            cum, j = lst[idx]
            cj = closure(j)
            if len(prefs) != idx:
                continue
            base = dict(prefs[-1]) if prefs else {}
            merge(base, cj)
            base[semid] = cum
            prefs.append(base)
        lo, hi = 0, len(lst)
        while lo < hi:
            mid = (lo + hi) // 2
            if lst[mid][0] <= v:
                lo = mid + 1
            else:
                hi = mid
        if lo > 0:
            merge(out, prefs[lo - 1])
        return out

    for attempt in range(3):
        closure_memo.clear()
        prefix_memo.clear()
        bad = _strip_pass(
            insts, sem_incs, merge, wait_implies, push_extras=(attempt == 2)
        )
        if not bad:
            return
    raise RuntimeError(
        f"instructions still have >1 sync wait after transitive "
        f"reduction: {bad[:5]} ({len(bad)} total)"
    )


def _strip_pass(insts, sem_incs, merge, wait_implies, push_extras):
    bad = []
    for k, ins in enumerate(insts):
        limit = 1
        si = ins.sync_info
        if si is None or len(si.on_wait) <= limit:
            continue
        waits = list(si.on_wait)
        changed = True
        while len(waits) > 1 and changed:
            changed = False
            for wi, w in enumerate(waits):
                if not (
                    w.sync_type == "semaphore"
                    and w.wait_mode == "sem-ge-imm"
                    and w.wait_reg is None
                ):
                    continue
                implied = {}
                for wj, w2 in enumerate(waits):
                    if wj == wi:
                        continue
                    if (
                        w2.sync_type == "semaphore"
                        and w2.wait_mode == "sem-ge-imm"
                        and w2.wait_reg is None
                    ):
                        merge(implied, wait_implies(w2.id, w2.wait_value))
                if implied.get(w.id, -1) >= w.wait_value:
                    waits.pop(wi)
                    changed = True
                    break
        if len(waits) > limit and push_extras:
            # Fallback: push extra waits onto earlier same-engine
            # instructions.  Safe when every increment satisfying the wait
            # sits earlier in the (topologically ordered) schedule than the
            # target instruction, so the moved wait cannot deadlock.
            def last_incrementer_pos(w):
                lst = sem_incs.get(w.id, [])
                pos = -1
                for cum, j in lst:
                    if cum > w.wait_value:
                        break
                    pos = max(pos, j)
                return pos

            waits.sort(key=last_incrementer_pos)
            keep = waits[-limit:]
            extras = waits[:-limit]
            eng = ins.engine.name
            kprev = k - 1
            while extras and kprev >= 0:
                cand = insts[kprev]
                csi = cand.sync_info
                if (
                    cand.engine.name == eng
                    and csi is not None
                    and len(csi.on_wait) == 0
                ):
                    w = extras[-1]
                    if last_incrementer_pos(w) < kprev:
                        extras.pop()
                        csi.on_wait = [w]
                        cand.sync_info = csi
                kprev -= 1
            waits = extras + keep
        if len(waits) > limit:
            bad.append((ins.name, [(w.ant_name, w.wait_value) for w in waits]))
        if len(waits) != len(si.on_wait):
            si.on_wait = waits
            ins.sync_info = si
    return bad


def _get_program(n, m, wire_dt=WIRE_DT, debug=False):
    key = (n, m, wire_dt, debug)
    if key not in _PROG_CACHE:
        _PROG_CACHE[key] = _build_program(n, m, wire_dt, debug)
    return _PROG_CACHE[key]


def _l2norm(x):
    nrm = np.sqrt((x * x).sum(axis=-1, keepdims=True))
    return x / np.maximum(nrm, EPS)


def _host_inputs_batch(xyz1, n1, xyz2, n2, sq1, sq2, n, m, np_dt):
    """[2B, 9, n+m] upload for cores (b,dir): each batch's transposed point
    set is converted to the wire dtype once and written to both cores."""
    b = xyz1.shape[0]
    ab = np.empty((2 * b, 9, n + m), np_dt)
    for bi in range(b):
        p1t = np.empty((6, n), np_dt)
        p1t[0:3] = xyz1[bi].T
        p1t[3:6] = n1[bi].T
        p2t = np.empty((6, m), np_dt)
        p2t[0:3] = xyz2[bi].T
        p2t[3:6] = n2[bi].T
        h1 = (sq1[bi] * 0.5).astype(np_dt)
        h2 = (sq2[bi] * 0.5).astype(np_dt)
        for d in range(2):
            a = ab[2 * bi + d]
            a[0:6, 0:n] = p1t if d == 0 else p2t
            a[6, 0:n] = h1 if d == 0 else h2
            a[7, 0:n] = 1.0
            a[8, 0:n] = 0.0
            a[0:6, n:] = p2t if d == 0 else p1t
            a[6, n:] = -1.0
            np.negative(h2 if d == 0 else h1, out=a[7, n:])
            a[8, n:] = 1.0
    return ab


_LAST_RUN_INFO = {}
_RUNNER_CACHE = {}


def _get_runner(n, m, n_cores, wire_dt=WIRE_DT, debug=False):
    """Build (once) a persistent jitted SPMD executor for the program.

    Output buffers are created inside the jit (the kernel writes every
    byte of the output tensor), so only `ab` crosses the axon tunnel.
    """
    key = (n, m, n_cores, wire_dt, debug)
    if key in _RUNNER_CACHE:
        return _RUNNER_CACHE[key]

    import jax
    import jax.numpy as jnp
    from jax.experimental.shard_map import shard_map
    from jax.sharding import Mesh, PartitionSpec

    from concourse import bass2jax, mybir

    nc = _get_program(n, m, wire_dt, debug)
    bass2jax.install_neuronx_cc_hook()

    partition_name = (
        nc.partition_id_tensor.name if nc.partition_id_tensor else None
    )
    in_names, out_names, out_avals = [], [], []
    for alloc in nc.m.functions[0].allocations:
        if not isinstance(alloc, mybir.MemoryLocationSet):
            continue
        name = alloc.memorylocations[0].name
        if alloc.kind == "ExternalInput":
            if name != partition_name:
                in_names.append(name)
        elif alloc.kind == "ExternalOutput":
            out_names.append(name)
            shape = tuple(alloc.tensor_shape)
            dtype = mybir.dt.np(alloc.dtype)
            out_avals.append(jax.core.ShapedArray(shape, dtype))
    in_names_all = list(in_names) + list(out_names)
    if partition_name is not None:
        in_names_all.append(partition_name)

    def _body(*args):
        operands = list(args)
        if partition_name is not None:
            operands.append(bass2jax.partition_id_tensor())
        outs = bass2jax._bass_exec_p.bind(
            *operands,
            out_avals=tuple(out_avals),
            in_names=tuple(in_names_all),
            out_names=tuple(out_names),
            lowering_input_output_aliases=(),
            sim_require_finite=True,
            sim_require_nnan=True,
            nc=nc,
        )
        return tuple(outs)

    n_params = len(in_names)
    n_outs = len(out_avals)
    devices = jax.devices()[:n_cores]
    mesh = Mesh(np.asarray(devices), ("core",))
    sharded = jax.jit(
        shard_map(
            _body,
            mesh=mesh,
            in_specs=(PartitionSpec("core"),) * (n_params + n_outs),
            out_specs=(PartitionSpec("core"),) * n_outs,
            check_rep=False,
        ),
        keep_unused=True,
    )

    # Zero "output" operands: the checker requires real jit parameters, but
    # the kernel writes every output byte, so ship them to the devices ONCE
    # and reuse (not donated, hence never consumed).
    from jax.sharding import NamedSharding

    sh = NamedSharding(mesh, PartitionSpec("core"))
    dev_zeros = [
        jax.device_put(
            np.zeros((n_cores * av.shape[0], *av.shape[1:]), av.dtype), sh
        )
        for av in out_avals
    ]

    runner = {
        "sharded": sharded,
        "in_names": in_names,
        "out_names": out_names,
        "out_avals": out_avals,
        "dev_zeros": dev_zeros,
        "n_cores": n_cores,
    }
    _RUNNER_CACHE[key] = runner
    return runner


def _run_jobs(ab_batch, n, m, wire_dt=WIRE_DT, debug=False):
    """ab_batch: [n_cores, 9, n+m] in the wire dtype, or a list of per-core
    {"ab": [9, n+m]} maps (concatenated here)."""
    import time

    if isinstance(ab_batch, list):
        n_cores = len(ab_batch)
        ab_batch = np.concatenate(
            [m_["ab"][None] for m_ in ab_batch], axis=0
        ).reshape(n_cores * 9, n + m)
    n_cores = ab_batch.shape[0] if ab_batch.ndim == 3 else ab_batch.shape[0] // 9
    r = _get_runner(n, m, n_cores, wire_dt, debug)
    concat_in = [ab_batch.reshape(n_cores * 9, n + m)]
    t0 = time.time()
    out_arrs = r["sharded"](*concat_in, *r["dev_zeros"])
    out_np = [np.asarray(a) for a in out_arrs]
    _LAST_RUN_INFO["exec_wall_ns"] = (time.time() - t0) * 1e9
    _LAST_RUN_INFO["exec_time_ns"] = None
    name_i = {name: i for i, name in enumerate(r["out_names"])}
    i = name_i["res"]
    av = r["out_avals"][i]
    per_core = out_np[i].reshape(n_cores, *av.shape)
    return [per_core[c] for c in range(n_cores)]


def kernel(xyz1, xyz2, normal_rebuild, normal_gt):
    xyz1 = np.asarray(xyz1, np.float32)
    xyz2 = np.asarray(xyz2, np.float32)
    normal_rebuild = np.asarray(normal_rebuild, np.float32)
    normal_gt = np.asarray(normal_gt, np.float32)
    b, n = xyz1.shape[0], xyz1.shape[1]
    m = xyz2.shape[1]
    np_dt = np.float32 if WIRE_DT == "f32" else np.float16

    n1 = _l2norm(normal_rebuild)
    n2 = _l2norm(normal_gt)
    # normals are unit vectors, so the 6D square norm is |xyz|^2 + 1
    sq1 = np.einsum("bnc,bnc->bn", xyz1, xyz1) + np.float32(1.0)
    sq2 = np.einsum("bnc,bnc->bn", xyz2, xyz2) + np.float32(1.0)

    jobs = []  # (q_xyz, q_n, qsq, db_xyz, db_n, dbsq)
    for core in range(2 * b):
        bi, d = core // 2, core % 2
        if d == 0:
            job = (xyz1[bi], n1[bi], sq1[bi], xyz2[bi], n2[bi], sq2[bi])
        else:
            job = (xyz2[bi], n2[bi], sq2[bi], xyz1[bi], n1[bi], sq1[bi])
        jobs.append(job)
    ab_batch = _host_inputs_batch(xyz1, n1, xyz2, n2, sq1, sq2, n, m, np_dt)

    outs = _run_jobs(ab_batch, n, m)

    xyz_sums = [0.0, 0.0]
    nrm_sums = [0.0, 0.0]
    counts = [0, 0]
    nbq = n // P
    for core, raw in enumerate(outs):
        d = core % 2
        q_xyz, q_n, qsq, db_xyz, db_n, dbsq = jobs[core]
        cnt = raw[:, 0:nbq].T.reshape(-1)  # query qb*128+p at [p, qb]
        xyz_sum = float(raw[0, nbq])
        nrm_sum = float(raw[0, nbq + 1])
        # same f32 predicate the device used, bitwise
        d1 = (cnt - np.float32(1.0)).astype(np.float32)
        bad = np.nonzero(~((d1 * d1) <= np.float32(0.0025)))[0]
        if bad.size:
            # exact host recompute (vectorized): ties / out-of-band rows
            q6b = np.concatenate([q_xyz[bad], q_n[bad]], axis=1)
            db6 = np.concatenate([db_xyz, db_n], axis=1)
            dbad = (
                qsq[bad][:, None]
                + dbsq[None, :]
                - 2.0 * (q6b @ db6.T)
            )
            j = np.argmin(dbad, axis=1)
            t_xyz = db_xyz[j]
            t_n = db_n[j]
            xyz_sum += float(((q_xyz[bad] - t_xyz) ** 2).sum())
            a = _l2norm(q_n[bad])
            tn = _l2norm(t_n)
            nrm_sum += float(np.minimum(
                ((a - tn) ** 2).sum(axis=1), ((a + tn) ** 2).sum(axis=1)
            ).sum())
        xyz_sums[d] += xyz_sum
        nrm_sums[d] += nrm_sum
        counts[d] += n

    xyz_out = xyz_sums[0] / counts[0] + xyz_sums[1] / counts[1]
    nrm_out = nrm_sums[0] / counts[0] + nrm_sums[1] / counts[1]
    return (np.float32(xyz_out), np.float32(nrm_out))

